# revision 7
# baseline (speedup 1.0000x reference)
"""DiffusionStep kernel v4: gather-only design, no scatter (d=4 gather).

Host assigns each node's edges round-robin over 8 cores (<=16 per
(node, core)) and lays each core's edge stream out in node-slot order:
node n owns columns [16*(n>>3), 16*(n>>3)+16) of partition-group
g = n % 8.  Device: ap_gather (d=4, 64-node blocks) fetches 4 x
candidates per slot, DVE mask (dst%64 == 4*(p%16)+s) selects,
ones-matmul contracts the 16 rows of each group, one tensor_reduce
folds the 4 s-slots -> per-edge messages at node-sorted (g, column)
positions, multiply by w, tensor_reduce 16->1 -> per-core partials
[8, 12544] streamed to DRAM, AllReduce, affine relayout to out[N, 1].
"""
import sys

sys.path.insert(0, "/opt/trn_rl_repo")

import numpy as np

N = 100000
E = 6400000
NCORES = 8
G = 8
D = 4                      # gather candidates per slot
NB = 64                    # nodes per gather block (16 rows * D)
NE = 1563                  # gather num_elems: ceil(N / NB)
NPAD = NE * NB             # 100032
CH = 2048                  # columns (edges per Q7 core) per chunk
NCH_FULL = 98              # chunks: 98*2048 = 200704 columns per group
J = CH * NCH_FULL
NPG = 12544                # node slots per group (= J / 16)
OGRP = 8                   # chunks per outacc->DRAM flush

_cache = {}


def _build(nch, passes=1):
    from concourse import bacc, mybir, tile

    nc = bacc.Bacc(None, target_bir_lowering=False)
    f32, i16, i8 = mybir.dt.float32, mybir.dt.int16, mybir.dt.int8

    j = CH * nch
    npg = j // 16
    dstw = nc.declare_dram_parameter("dstw", [128, j // 16], i16, isOutput=False)
    dlow = nc.declare_dram_parameter("dlow", [G, j], i8, isOutput=False)
    wst = nc.declare_dram_parameter("wst", [G, j], f32, isOutput=False)
    xtab = nc.declare_dram_parameter("xtab", [128, D * NE], f32, isOutput=False)
    iota4 = nc.declare_dram_parameter("iota4", [128, D], f32, isOutput=False)
    bones = nc.declare_dram_parameter("bones", [128, G], f32, isOutput=False)
    out = nc.declare_dram_parameter("out", [N, 1], f32, isOutput=True)

    partial = nc.dram_tensor("partial", [G, npg], f32)
    reduced = nc.dram_tensor("reduced", [G, npg], f32, addr_space="Shared")

    with tile.TileContext(nc) as tc:
        with tc.tile_pool(name="const", bufs=1) as cpool, \
             tc.tile_pool(name="work", bufs=2) as pool, \
             tc.tile_pool(name="ps", bufs=2, space="PSUM") as pspool:

            xtab_sb = cpool.tile([128, D * NE], f32, tag="xtab")
            nc.sync.dma_start(out=xtab_sb[:], in_=xtab[:])
            iota4_sb = cpool.tile([128, D], f32, tag="iota4")
            nc.sync.dma_start(out=iota4_sb[:], in_=iota4[:])
            bones_sb = cpool.tile([128, G], f32, tag="bones")
            nc.sync.dma_start(out=bones_sb[:], in_=bones[:])

            with tc.For_i(0, passes):
                outacc = None
                for ch in range(nch):
                    csl = slice(ch * CH, (ch + 1) * CH)
                    wsl = slice(ch * (CH // 16), (ch + 1) * (CH // 16))
                    if ch % OGRP == 0:
                        ogrp_n = min(OGRP, nch - ch)
                        outacc = pool.tile([G, ogrp_n * (CH // 16)], f32,
                                           tag="outacc")

                    # gather D candidates per slot from the edge's 64-block
                    idx_c = pool.tile([128, CH // 16], i16, tag="idx_c")
                    nc.sync.dma_start(out=idx_c[:], in_=dstw[:, wsl])
                    cand = pool.tile([128, D * CH], f32, tag="cand")
                    nc.gpsimd.ap_gather(
                        out_ap=cand[:], in_ap=xtab_sb[:], idxs_ap=idx_c[:],
                        channels=128, num_elems=NE, d=D, num_idxs=CH,
                    )

                    # mask: keep cand s at row p iff dst%64 == 4*(p%16)+s
                    dl8 = pool.tile([128, CH], i8, tag="dl8")
                    nc.sync.dma_start(
                        out=dl8[:],
                        in_=dlow[:, csl].unsqueeze(1).to_broadcast([G, 16, CH]),
                    )
                    mask = pool.tile([128, D * CH], f32, tag="mask")
                    nc.vector.tensor_tensor(
                        out=mask[:].rearrange("p (j s) -> p j s", s=D),
                        in0=dl8[:].unsqueeze(2).to_broadcast([128, CH, D]),
                        in1=iota4_sb[:].unsqueeze(1).to_broadcast([128, CH, D]),
                        op=mybir.AluOpType.is_equal,
                    )
                    nc.vector.tensor_tensor(
                        out=cand[:], in0=cand[:], in1=mask[:],
                        op=mybir.AluOpType.mult,
                    )

                    # contract 16 rows per group (4 matmuls per PSUM tile),
                    # fold the D s-slots in one reduce, weights, 16 -> 1
                    w_c = pool.tile([G, CH], f32, tag="w_c")
                    nc.sync.dma_start(out=w_c[:], in_=wst[:, csl])
                    sb8 = pool.tile([G, CH], f32, tag="sb8")
                    for q in range(4):
                        ps = pspool.tile([G, CH], f32, tag="ps")
                        for k in range(4):
                            off = q * CH + k * 512
                            nc.tensor.matmul(
                                out=ps[:, k * 512:(k + 1) * 512],
                                lhsT=bones_sb[:],
                                rhs=cand[:, off:off + 512],
                                start=True, stop=True,
                            )
                        nc.vector.tensor_reduce(
                            out=sb8[:, q * 512:(q + 1) * 512],
                            in_=ps[:].rearrange("g (j s) -> g j s", s=D),
                            axis=mybir.AxisListType.X, op=mybir.AluOpType.add,
                        )
                    nc.vector.tensor_tensor(
                        out=sb8[:], in0=sb8[:], in1=w_c[:],
                        op=mybir.AluOpType.mult,
                    )
                    ob = (ch % OGRP) * (CH // 16)
                    nc.vector.tensor_reduce(
                        out=outacc[:, ob:ob + CH // 16],
                        in_=sb8[:].rearrange("g (q i) -> g q i", i=16),
                        axis=mybir.AxisListType.X, op=mybir.AluOpType.add,
                    )
                    if ch % OGRP == OGRP - 1 or ch == nch - 1:
                        base = (ch - ch % OGRP) * (CH // 16)
                        nc.sync.dma_start(
                            out=partial[:, base:base + (ch % OGRP + 1) * (CH // 16)],
                            in_=outacc[:, 0:(ch % OGRP + 1) * (CH // 16)],
                        )

            # combine across cores
            nc.gpsimd.collective_compute(
                "AllReduce",
                mybir.AluOpType.add,
                replica_groups=[list(range(NCORES))],
                ins=[partial[:]],
                outs=[reduced[:]],
            )
            # out[n] = reduced[n % 8, n >> 3]
            with nc.allow_non_contiguous_dma(reason="final relayout"):
                nc.sync.dma_start(
                    out=out[0:N, 0].rearrange("(q g) -> g q", g=G),
                    in_=reduced[:, 0:N // G],
                )

    nc.finalize()
    return nc


def _get_nc(nch):
    if nch not in _cache:
        _cache[nch] = _build(nch)
    return _cache[nch]


def _build_timing(nch, passes=2):
    key = (nch, passes)
    if key not in _cache:
        _cache[key] = _build(nch, passes=passes)
    return _cache[key]


def _host_prep(x, edge_index, edge_weight, nch):
    x = np.asarray(x, dtype=np.float32).reshape(N)
    ei = np.asarray(edge_index)
    src = ei[0].astype(np.int64)
    dst = ei[1].astype(np.int32)
    w = np.asarray(edge_weight, dtype=np.float32)
    j = CH * nch

    # sort by src; rank within node; round-robin cores; node-slot columns
    order = np.argsort(src, kind="stable")
    s_s, d_s, w_s = src[order], dst[order], w[order]
    first = np.zeros(N + 1, np.int64)
    np.add.at(first, s_s + 1, 1)
    starts = np.cumsum(first)[:-1]
    rank = np.arange(E, dtype=np.int64) - starts[s_s]
    core = (rank % NCORES).astype(np.int32)
    i16r = rank // NCORES
    assert i16r.max() < 16, f"per-(node,core) overflow: {i16r.max()}"
    g = (s_s % G).astype(np.int32)
    col = (s_s >> 3) * 16 + i16r
    assert col.max() < j

    xpad = np.zeros(NPAD, np.float32)
    xpad[:N] = x
    xtab = np.tile(
        np.ascontiguousarray(
            xpad.reshape(NE, 16, D).transpose(1, 0, 2).reshape(16, D * NE)
        ),
        (8, 1),
    )
    iota4 = (D * (np.arange(128)[:, None] % 16)
             + np.arange(D)[None, :]).astype(np.float32)
    bones = np.zeros((128, G), np.float32)
    for gg in range(G):
        bones[16 * gg:16 * (gg + 1), gg] = 1.0

    in_maps = []
    for c in range(NCORES):
        m = core == c
        dc = np.zeros((G, j), np.int32)
        wc = np.zeros((G, j), np.float32)
        dc[g[m], col[m]] = d_s[m]
        wc[g[m], col[m]] = w_s[m]
        idx = (dc >> 6).astype(np.int16)            # [G, j]
        # wrap: dstw[16*gg + jj%16, jj//16] = idx[gg, jj]
        dstw = np.ascontiguousarray(
            idx.reshape(G, j // 16, 16).transpose(0, 2, 1)
        ).reshape(128, j // 16)
        in_maps.append({
            "dstw": dstw,
            "dlow": np.ascontiguousarray((dc & 63).astype(np.int8)),
            "wst": np.ascontiguousarray(wc),
            "xtab": xtab,
            "iota4": iota4,
            "bones": bones,
        })
    return in_maps


def kernel(x, edge_index, edge_weight, nch=NCH_FULL):
    from concourse.bass_utils import run_bass_kernel_spmd

    nc = _get_nc(nch)
    in_maps = _host_prep(x, edge_index, edge_weight, nch)
    res = run_bass_kernel_spmd(nc, in_maps, list(range(NCORES)))
    out = res.results[0]["out"].astype(np.float32).reshape(N, 1)
    return out


# revision 8
# speedup vs baseline: 1.9413x; 1.9413x over previous
"""DiffusionStep kernel v5: gather-only design, no scatter (d=2 gather).

Host assigns each node's edges round-robin over 8 cores (<=16 per
(node, core)) and lays each core's edge stream out in node-slot order:
node n owns columns [16*(n>>3), 16*(n>>3)+16) of partition-group
g = n % 8.  Device: ap_gather (d=2, 32-node blocks) fetches 2 x
candidates per slot, DVE mask (dst%32 == 2*(p%16)+s) selects,
ones-matmul contracts the 16 rows of each group, one tensor_reduce
folds the 2 s-slots -> per-edge messages at node-sorted (g, column)
positions, multiply by w, tensor_reduce 16->1 -> per-core partials
[8, 12544], AllReduce, affine relayout to out[N, 1].
"""
import sys

sys.path.insert(0, "/opt/trn_rl_repo")

import numpy as np

N = 100000
E = 6400000
NCORES = 8
G = 8
CH = 2048                  # columns (edges per Q7 core) per chunk
NCH_FULL = 98              # chunks: 98*2048 = 200704 columns per group
J = CH * NCH_FULL
NPG = 12544                # node slots per group (= J / 16)
NE = 3125                  # ap_gather num_elems (32-node blocks)

_cache = {}


def _build(nch, passes=1):
    from concourse import bacc, mybir, tile

    nc = bacc.Bacc(None, target_bir_lowering=False)
    f32, bf16, i16, i8 = (mybir.dt.float32, mybir.dt.bfloat16,
                          mybir.dt.int16, mybir.dt.int8)

    j = CH * nch
    npg = j // 16
    dstw = nc.declare_dram_parameter("dstw", [128, j // 16], i16, isOutput=False)
    dlow = nc.declare_dram_parameter("dlow", [G, j], i8, isOutput=False)
    wst = nc.declare_dram_parameter("wst", [G, j], f32, isOutput=False)
    xtab = nc.declare_dram_parameter("xtab", [128, 2 * NE], f32, isOutput=False)
    iota2 = nc.declare_dram_parameter("iota2", [128, 2], f32, isOutput=False)
    bones = nc.declare_dram_parameter("bones", [128, G], f32, isOutput=False)
    out = nc.declare_dram_parameter("out", [N, 1], f32, isOutput=True)

    partial = nc.dram_tensor("partial", [G, npg], f32)
    reduced = nc.dram_tensor("reduced", [G, npg], f32, addr_space="Shared")

    with tile.TileContext(nc) as tc:
        with tc.tile_pool(name="const", bufs=1) as cpool, \
             tc.tile_pool(name="work", bufs=2) as pool, \
             tc.tile_pool(name="acc", bufs=1) as apool, \
             tc.tile_pool(name="ps", bufs=2, space="PSUM") as pspool:

            xtab_sb = cpool.tile([128, 2 * NE], f32, tag="xtab")
            nc.sync.dma_start(out=xtab_sb[:], in_=xtab[:])
            iota2_sb = cpool.tile([128, 2], f32, tag="iota2")
            nc.sync.dma_start(out=iota2_sb[:], in_=iota2[:])
            bones_sb = cpool.tile([128, G], f32, tag="bones")
            nc.sync.dma_start(out=bones_sb[:], in_=bones[:])

            outacc = apool.tile([G, npg], f32, tag="outacc")

            with tc.For_i(0, passes):
                for ch in range(nch):
                    csl = slice(ch * CH, (ch + 1) * CH)
                    wsl = slice(ch * (CH // 16), (ch + 1) * (CH // 16))

                    # gather 2 candidates per slot from the edge's 32-block
                    idx_c = pool.tile([128, CH // 16], i16, tag="idx_c")
                    nc.scalar.dma_start(out=idx_c[:], in_=dstw[:, wsl])
                    cand = pool.tile([128, 2 * CH], f32, tag="cand")
                    nc.gpsimd.ap_gather(
                        out_ap=cand[:], in_ap=xtab_sb[:], idxs_ap=idx_c[:],
                        channels=128, num_elems=NE, d=2, num_idxs=CH,
                    )

                    # mask: keep candidate s at row p iff dst%32 == 2*(p%16)+s
                    dl8 = pool.tile([128, CH], i8, tag="dl8")
                    nc.sync.dma_start(
                        out=dl8[:],
                        in_=dlow[:, csl].unsqueeze(1).to_broadcast([G, 16, CH]),
                    )
                    mask = pool.tile([128, 2 * CH], bf16, tag="mask")
                    nc.vector.tensor_tensor(
                        out=mask[:].rearrange("p (j s) -> p j s", s=2),
                        in0=dl8[:].unsqueeze(2).to_broadcast([128, CH, 2]),
                        in1=iota2_sb[:].unsqueeze(1).to_broadcast([128, CH, 2]),
                        op=mybir.AluOpType.is_equal,
                    )
                    nc.vector.tensor_tensor(
                        out=cand[:], in0=cand[:], in1=mask[:],
                        op=mybir.AluOpType.mult,
                    )

                    # contract 16 rows per group (4 matmuls into one PSUM
                    # tile), fold s-pairs in one reduce, weights, 16 -> 1
                    w_c = pool.tile([G, CH], f32, tag="w_c")
                    nc.scalar.dma_start(out=w_c[:], in_=wst[:, csl])
                    sb8 = pool.tile([G, CH], f32, tag="sb8")
                    for h in range(2):
                        ps = pspool.tile([G, CH], f32, tag="ps")
                        for k in range(4):
                            off = h * CH + k * 512
                            nc.tensor.matmul(
                                out=ps[:, k * 512:(k + 1) * 512],
                                lhsT=bones_sb[:],
                                rhs=cand[:, off:off + 512],
                                start=True, stop=True,
                            )
                        nc.vector.tensor_reduce(
                            out=sb8[:, h * (CH // 2):(h + 1) * (CH // 2)],
                            in_=ps[:].rearrange("g (j s) -> g j s", s=2),
                            axis=mybir.AxisListType.X, op=mybir.AluOpType.add,
                        )
                    nc.vector.tensor_tensor(
                        out=sb8[:], in0=sb8[:], in1=w_c[:],
                        op=mybir.AluOpType.mult,
                    )
                    nc.vector.tensor_reduce(
                        out=outacc[:, ch * (CH // 16):(ch + 1) * (CH // 16)],
                        in_=sb8[:].rearrange("g (q i) -> g q i", i=16),
                        axis=mybir.AxisListType.X, op=mybir.AluOpType.add,
                    )

            # combine across cores
            nc.sync.dma_start(out=partial[:], in_=outacc[:])
            nc.gpsimd.collective_compute(
                "AllReduce",
                mybir.AluOpType.add,
                replica_groups=[list(range(NCORES))],
                ins=[partial[:]],
                outs=[reduced[:]],
            )
            # out[n] = reduced[n % 8, n >> 3]
            with nc.allow_non_contiguous_dma(reason="final relayout"):
                nc.sync.dma_start(
                    out=out[0:N, 0].rearrange("(q g) -> g q", g=G),
                    in_=reduced[:, 0:N // G],
                )

    nc.finalize()
    return nc


def _get_nc(nch):
    if nch not in _cache:
        _cache[nch] = _build(nch)
    return _cache[nch]


def _build_timing(nch, passes=2):
    key = (nch, passes)
    if key not in _cache:
        _cache[key] = _build(nch, passes=passes)
    return _cache[key]


def _host_prep(x, edge_index, edge_weight, nch):
    x = np.asarray(x, dtype=np.float32).reshape(N)
    ei = np.asarray(edge_index)
    src = ei[0].astype(np.int64)
    dst = ei[1].astype(np.int32)
    w = np.asarray(edge_weight, dtype=np.float32)
    j = CH * nch

    # sort by src; rank within node; round-robin cores; node-slot columns
    order = np.argsort(src, kind="stable")
    s_s, d_s, w_s = src[order], dst[order], w[order]
    first = np.zeros(N + 1, np.int64)
    np.add.at(first, s_s + 1, 1)
    starts = np.cumsum(first)[:-1]
    rank = np.arange(E, dtype=np.int64) - starts[s_s]
    core = (rank % NCORES).astype(np.int32)
    i16r = rank // NCORES
    assert i16r.max() < 16, f"per-(node,core) overflow: {i16r.max()}"
    g = (s_s % G).astype(np.int32)
    col = (s_s >> 3) * 16 + i16r
    assert col.max() < j

    xtab = np.tile(
        np.ascontiguousarray(
            x.reshape(NE, 16, 2).transpose(1, 0, 2).reshape(16, 2 * NE)
        ),
        (8, 1),
    )
    iota2 = np.stack(
        [2 * (np.arange(128) % 16), 2 * (np.arange(128) % 16) + 1], axis=1
    ).astype(np.float32)
    bones = np.zeros((128, G), np.float32)
    for gg in range(G):
        bones[16 * gg:16 * (gg + 1), gg] = 1.0

    in_maps = []
    for c in range(NCORES):
        m = core == c
        dc = np.zeros((G, j), np.int32)
        wc = np.zeros((G, j), np.float32)
        dc[g[m], col[m]] = d_s[m]
        wc[g[m], col[m]] = w_s[m]
        idx = (dc >> 5).astype(np.int16)            # [G, j]
        # wrap: dstw[16*gg + jj%16, jj//16] = idx[gg, jj]
        dstw = np.ascontiguousarray(
            idx.reshape(G, j // 16, 16).transpose(0, 2, 1)
        ).reshape(128, j // 16)
        in_maps.append({
            "dstw": dstw,
            "dlow": np.ascontiguousarray((dc & 31).astype(np.int8)),
            "wst": np.ascontiguousarray(wc),
            "xtab": xtab,
            "iota2": iota2,
            "bones": bones,
        })
    return in_maps


def kernel(x, edge_index, edge_weight, nch=NCH_FULL):
    from concourse.bass_utils import run_bass_kernel_spmd

    nc = _get_nc(nch)
    in_maps = _host_prep(x, edge_index, edge_weight, nch)
    res = run_bass_kernel_spmd(nc, in_maps, list(range(NCORES)))
    out = res.results[0]["out"].astype(np.float32).reshape(N, 1)
    return out


# revision 10
# speedup vs baseline: 2.3916x; 1.2320x over previous
"""DiffusionStep kernel v5: gather-only design, no scatter (d=2 gather).

Host assigns each node's edges round-robin over 8 cores (<=16 per
(node, core)) and lays each core's edge stream out in node-slot order:
node n owns columns [16*(n>>3), 16*(n>>3)+16) of partition-group
g = n % 8.  Device: ap_gather (d=2, 32-node blocks) fetches 2 x
candidates per slot, DVE mask (dst%32 == 2*(p%16)+s) selects,
ones-matmul contracts the 16 rows of each group, one tensor_reduce
folds the 2 s-slots -> per-edge messages at node-sorted (g, column)
positions, multiply by w, tensor_reduce 16->1 -> per-core partials
[8, 12544], AllReduce, affine relayout to out[N, 1].
"""
import sys

sys.path.insert(0, "/opt/trn_rl_repo")

import numpy as np

N = 100000
E = 6400000
NCORES = 8
G = 8
SLOT = 12                  # padded edge slots per (node, core)
CH = 2016                  # columns (edges per Q7 core) per chunk
NCH_FULL = 75              # chunks: 75*2016 = 151200 columns per group
J = CH * NCH_FULL
NPG = J // SLOT            # 12600 node slots per group (12500 real)
NREAL = 12500              # real nodes per group
NE = 3125                  # ap_gather num_elems (32-node blocks)

_cache = {}


def _build(nch, passes=1):
    from concourse import bacc, mybir, tile

    nc = bacc.Bacc(None, target_bir_lowering=False)
    f32, bf16, i16, i8 = (mybir.dt.float32, mybir.dt.bfloat16,
                          mybir.dt.int16, mybir.dt.int8)

    j = CH * nch
    npg = j // SLOT
    dstw = nc.declare_dram_parameter("dstw", [128, j // 16], i16, isOutput=False)
    dlow = nc.declare_dram_parameter("dlow", [G, j], i8, isOutput=False)
    wst = nc.declare_dram_parameter("wst", [G, j], f32, isOutput=False)
    xtab = nc.declare_dram_parameter("xtab", [128, 2 * NE], f32, isOutput=False)
    iota2 = nc.declare_dram_parameter("iota2", [128, 2], f32, isOutput=False)
    bones = nc.declare_dram_parameter("bones", [128, G], f32, isOutput=False)
    out = nc.declare_dram_parameter("out", [N, 1], f32, isOutput=True)
    vout = nc.declare_dram_parameter("vout", [G, NPG - NREAL], f32, isOutput=True)

    partial = nc.dram_tensor("partial", [G, npg], f32)
    reduced = nc.dram_tensor("reduced", [G, npg], f32, addr_space="Shared")

    with tile.TileContext(nc) as tc:
        with tc.tile_pool(name="const", bufs=1) as cpool, \
             tc.tile_pool(name="work", bufs=2) as pool, \
             tc.tile_pool(name="acc", bufs=1) as apool, \
             tc.tile_pool(name="ps", bufs=2, space="PSUM") as pspool:

            xtab_sb = cpool.tile([128, 2 * NE], f32, tag="xtab")
            nc.sync.dma_start(out=xtab_sb[:], in_=xtab[:])
            iota2_sb = cpool.tile([128, 2], f32, tag="iota2")
            nc.sync.dma_start(out=iota2_sb[:], in_=iota2[:])
            bones_sb = cpool.tile([128, G], f32, tag="bones")
            nc.sync.dma_start(out=bones_sb[:], in_=bones[:])

            outacc = apool.tile([G, npg], f32, tag="outacc")

            with tc.For_i(0, passes):
                for ch in range(nch):
                    csl = slice(ch * CH, (ch + 1) * CH)
                    wsl = slice(ch * (CH // 16), (ch + 1) * (CH // 16))

                    # gather 2 candidates per slot from the edge's 32-block
                    idx_c = pool.tile([128, CH // 16], i16, tag="idx_c")
                    nc.scalar.dma_start(out=idx_c[:], in_=dstw[:, wsl])
                    cand = pool.tile([128, 2 * CH], f32, tag="cand")
                    nc.gpsimd.ap_gather(
                        out_ap=cand[:], in_ap=xtab_sb[:], idxs_ap=idx_c[:],
                        channels=128, num_elems=NE, d=2, num_idxs=CH,
                    )

                    # mask: keep candidate s at row p iff dst%32 == 2*(p%16)+s
                    dl8 = pool.tile([128, CH], i8, tag="dl8")
                    nc.sync.dma_start(
                        out=dl8[:],
                        in_=dlow[:, csl].unsqueeze(1).to_broadcast([G, 16, CH]),
                    )
                    mask = pool.tile([128, 2 * CH], bf16, tag="mask")
                    nc.vector.tensor_tensor(
                        out=mask[:].rearrange("p (j s) -> p j s", s=2),
                        in0=dl8[:].unsqueeze(2).to_broadcast([128, CH, 2]),
                        in1=iota2_sb[:].unsqueeze(1).to_broadcast([128, CH, 2]),
                        op=mybir.AluOpType.is_equal,
                    )
                    nc.vector.tensor_tensor(
                        out=cand[:], in0=cand[:], in1=mask[:],
                        op=mybir.AluOpType.mult,
                    )

                    # contract 16 rows per group (4 matmuls into one PSUM
                    # tile), fold s-pairs in one reduce, weights, 16 -> 1
                    w_c = pool.tile([G, CH], f32, tag="w_c")
                    nc.scalar.dma_start(out=w_c[:], in_=wst[:, csl])
                    sb8 = pool.tile([G, CH], f32, tag="sb8")
                    for h in range(2):
                        # padded to 2048 so each matmul slice starts on a
                        # 2KB PSUM bank boundary (outputs must not cross)
                        ps = pspool.tile([G, 2048], f32, tag="ps")
                        for k in range(4):
                            width = 512 if k < 3 else CH - 3 * 512
                            off = h * CH + k * 512
                            nc.tensor.matmul(
                                out=ps[:, k * 512:k * 512 + width],
                                lhsT=bones_sb[:],
                                rhs=cand[:, off:off + width],
                                start=True, stop=True,
                            )
                        nc.vector.tensor_reduce(
                            out=sb8[:, h * (CH // 2):(h + 1) * (CH // 2)],
                            in_=ps[:, 0:CH].rearrange("g (j s) -> g j s", s=2),
                            axis=mybir.AxisListType.X, op=mybir.AluOpType.add,
                        )
                    nc.vector.tensor_tensor(
                        out=sb8[:], in0=sb8[:], in1=w_c[:],
                        op=mybir.AluOpType.mult,
                    )
                    nc.vector.tensor_reduce(
                        out=outacc[:, ch * (CH // SLOT):(ch + 1) * (CH // SLOT)],
                        in_=sb8[:].rearrange("g (q i) -> g q i", i=SLOT),
                        axis=mybir.AxisListType.X, op=mybir.AluOpType.add,
                    )

            # combine across cores
            nc.sync.dma_start(out=partial[:], in_=outacc[:])
            nc.gpsimd.collective_compute(
                "AllReduce",
                mybir.AluOpType.add,
                replica_groups=[list(range(NCORES))],
                ins=[partial[:]],
                outs=[reduced[:]],
            )
            # out[n] = reduced[n % 8, n >> 3]; virtual partials to vout
            with nc.allow_non_contiguous_dma(reason="final relayout"):
                nc.sync.dma_start(
                    out=out[0:N, 0].rearrange("(q g) -> g q", g=G),
                    in_=reduced[:, 0:N // G],
                )
                nc.sync.dma_start(out=vout[:], in_=reduced[:, NREAL:npg])

    nc.finalize()
    return nc


def _get_nc(nch):
    if nch not in _cache:
        _cache[nch] = _build(nch)
    return _cache[nch]


def _build_timing(nch, passes=2):
    key = (nch, passes)
    if key not in _cache:
        _cache[key] = _build(nch, passes=passes)
    return _cache[key]


def _host_prep(x, edge_index, edge_weight, nch):
    x = np.asarray(x, dtype=np.float32).reshape(N)
    ei = np.asarray(edge_index)
    src = ei[0].astype(np.int64)
    dst = ei[1].astype(np.int32)
    w = np.asarray(edge_weight, dtype=np.float32)
    j = CH * nch

    # sort by src; rank within node; round-robin cores; node-slot columns
    order = np.argsort(src, kind="stable")
    s_s, d_s, w_s = src[order], dst[order], w[order]
    first = np.zeros(N + 1, np.int64)
    np.add.at(first, s_s + 1, 1)
    starts = np.cumsum(first)[:-1]
    rank = np.arange(E, dtype=np.int64) - starts[s_s]
    core = (rank % NCORES).astype(np.int32)
    i16r = rank // NCORES
    g = (s_s % G).astype(np.int32)
    col = (s_s >> 3) * SLOT + i16r
    # spill edges beyond SLOT per (node, core) to per-core virtual nodes
    npg = j // SLOT
    spill = i16r >= SLOT
    n_sp = int(spill.sum())
    assert n_sp <= (npg - NREAL) * G, f"too many spills: {n_sp}"
    vids = np.arange(n_sp, dtype=np.int64)
    g[spill] = vids % G
    col[spill] = (NREAL + vids // G) * SLOT
    core[spill] = 0
    vsrc = s_s[spill.nonzero()[0]]
    assert col.max() < j

    xtab = np.tile(
        np.ascontiguousarray(
            x.reshape(NE, 16, 2).transpose(1, 0, 2).reshape(16, 2 * NE)
        ),
        (8, 1),
    )
    iota2 = np.stack(
        [2 * (np.arange(128) % 16), 2 * (np.arange(128) % 16) + 1], axis=1
    ).astype(np.float32)
    bones = np.zeros((128, G), np.float32)
    for gg in range(G):
        bones[16 * gg:16 * (gg + 1), gg] = 1.0

    _host_prep.vsrc = vsrc
    in_maps = []
    for c in range(NCORES):
        m = core == c
        dc = np.zeros((G, j), np.int32)
        wc = np.zeros((G, j), np.float32)
        dc[g[m], col[m]] = d_s[m]
        wc[g[m], col[m]] = w_s[m]
        idx = (dc >> 5).astype(np.int16)            # [G, j]
        # wrap: dstw[16*gg + jj%16, jj//16] = idx[gg, jj]
        dstw = np.ascontiguousarray(
            idx.reshape(G, j // 16, 16).transpose(0, 2, 1)
        ).reshape(128, j // 16)
        in_maps.append({
            "dstw": dstw,
            "dlow": np.ascontiguousarray((dc & 31).astype(np.int8)),
            "wst": np.ascontiguousarray(wc),
            "xtab": xtab,
            "iota2": iota2,
            "bones": bones,
        })
    return in_maps


def kernel(x, edge_index, edge_weight, nch=NCH_FULL):
    from concourse.bass_utils import run_bass_kernel_spmd

    nc = _get_nc(nch)
    in_maps = _host_prep(x, edge_index, edge_weight, nch)
    res = run_bass_kernel_spmd(nc, in_maps, list(range(NCORES)))
    out = res.results[0]["out"].astype(np.float32).reshape(N, 1)
    vout = res.results[0]["vout"].astype(np.float32)
    vsrc = _host_prep.vsrc
    vvals = vout.T.reshape(-1)[:len(vsrc)]
    np.add.at(out[:, 0], vsrc, vvals)
    return out


# revision 11
# speedup vs baseline: 2.8056x; 1.1731x over previous
"""DiffusionStep kernel v5: gather-only design, no scatter (d=2 gather).

Host assigns each node's edges round-robin over 8 cores (<=16 per
(node, core)) and lays each core's edge stream out in node-slot order:
node n owns columns [16*(n>>3), 16*(n>>3)+16) of partition-group
g = n % 8.  Device: ap_gather (d=2, 32-node blocks) fetches 2 x
candidates per slot, DVE mask (dst%32 == 2*(p%16)+s) selects,
ones-matmul contracts the 16 rows of each group, one tensor_reduce
folds the 2 s-slots -> per-edge messages at node-sorted (g, column)
positions, multiply by w, tensor_reduce 16->1 -> per-core partials
[8, 12544], AllReduce, affine relayout to out[N, 1].
"""
import sys

sys.path.insert(0, "/opt/trn_rl_repo")

import numpy as np

N = 100000
E = 6400000
NCORES = 8
G = 8
SLOT = 12                  # padded edge slots per (node, core)
CH = 2016                  # columns (edges per Q7 core) per chunk
NCH_FULL = 75              # chunks: 75*2016 = 151200 columns per group
J = CH * NCH_FULL
NPG = J // SLOT            # 12600 node slots per group (12500 real)
NREAL = 12500              # real nodes per group
NE = 3125                  # ap_gather num_elems (32-node blocks)

_cache = {}


def _build(nch, passes=1):
    from concourse import bacc, mybir, tile

    nc = bacc.Bacc(None, target_bir_lowering=False)
    f32, bf16, i16, i8 = (mybir.dt.float32, mybir.dt.bfloat16,
                          mybir.dt.int16, mybir.dt.int8)

    j = CH * nch
    npg = j // SLOT
    dstw = nc.declare_dram_parameter("dstw", [128, j // 16], i16, isOutput=False)
    dlow = nc.declare_dram_parameter("dlow", [G, j], i8, isOutput=False)
    wst = nc.declare_dram_parameter("wst", [G, j], f32, isOutput=False)
    xtab = nc.declare_dram_parameter("xtab", [128, 2 * NE], f32, isOutput=False)
    iota2 = nc.declare_dram_parameter("iota2", [128, 2], f32, isOutput=False)
    bones = nc.declare_dram_parameter("bones", [128, G], f32, isOutput=False)
    out = nc.declare_dram_parameter("out", [N, 1], f32, isOutput=True)
    vout = nc.declare_dram_parameter("vout", [G, NPG - NREAL], f32, isOutput=True)

    partial = nc.dram_tensor("partial", [G, npg], f32)
    reduced = nc.dram_tensor("reduced", [G, npg], f32, addr_space="Shared")

    with tile.TileContext(nc) as tc:
        with tc.tile_pool(name="const", bufs=1) as cpool, \
             tc.tile_pool(name="work", bufs=3) as pool, \
             tc.tile_pool(name="acc", bufs=1) as apool, \
             tc.tile_pool(name="ps", bufs=2, space="PSUM") as pspool:

            xtab_sb = cpool.tile([128, 2 * NE], f32, tag="xtab")
            nc.sync.dma_start(out=xtab_sb[:], in_=xtab[:])
            iota2_sb = cpool.tile([128, 2], f32, tag="iota2")
            nc.sync.dma_start(out=iota2_sb[:], in_=iota2[:])
            bones_sb = cpool.tile([128, G], f32, tag="bones")
            nc.sync.dma_start(out=bones_sb[:], in_=bones[:])

            outacc = apool.tile([G, npg], f32, tag="outacc")

            with tc.For_i(0, passes):
                for ch in range(nch):
                    csl = slice(ch * CH, (ch + 1) * CH)
                    wsl = slice(ch * (CH // 16), (ch + 1) * (CH // 16))

                    # gather 2 candidates per slot from the edge's 32-block
                    idx_c = pool.tile([128, CH // 16], i16, tag="idx_c")
                    nc.scalar.dma_start(out=idx_c[:], in_=dstw[:, wsl])
                    cand = pool.tile([128, 2 * CH], f32, tag="cand")
                    nc.gpsimd.ap_gather(
                        out_ap=cand[:], in_ap=xtab_sb[:], idxs_ap=idx_c[:],
                        channels=128, num_elems=NE, d=2, num_idxs=CH,
                    )

                    # mask: keep candidate s at row p iff dst%32 == 2*(p%16)+s
                    dl8 = pool.tile([128, CH], i8, tag="dl8")
                    nc.sync.dma_start(
                        out=dl8[:],
                        in_=dlow[:, csl].unsqueeze(1).to_broadcast([G, 16, CH]),
                    )
                    mask = pool.tile([128, 2 * CH], bf16, tag="mask")
                    nc.vector.tensor_tensor(
                        out=mask[:].rearrange("p (j s) -> p j s", s=2),
                        in0=dl8[:].unsqueeze(2).to_broadcast([128, CH, 2]),
                        in1=iota2_sb[:].unsqueeze(1).to_broadcast([128, CH, 2]),
                        op=mybir.AluOpType.is_equal,
                    )
                    nc.vector.tensor_tensor(
                        out=cand[:], in0=cand[:], in1=mask[:],
                        op=mybir.AluOpType.mult,
                    )

                    # contract 16 rows per group (4 matmuls into one PSUM
                    # tile), fold s-pairs in one reduce, weights, 16 -> 1
                    w_c = pool.tile([G, CH], f32, tag="w_c")
                    nc.scalar.dma_start(out=w_c[:], in_=wst[:, csl])
                    sb8 = pool.tile([G, CH], f32, tag="sb8")
                    for h in range(2):
                        # padded to 2048 so each matmul slice starts on a
                        # 2KB PSUM bank boundary (outputs must not cross)
                        ps = pspool.tile([G, 2048], f32, tag="ps")
                        for k in range(4):
                            width = 512 if k < 3 else CH - 3 * 512
                            off = h * CH + k * 512
                            nc.tensor.matmul(
                                out=ps[:, k * 512:k * 512 + width],
                                lhsT=bones_sb[:],
                                rhs=cand[:, off:off + width],
                                start=True, stop=True,
                            )
                        nc.vector.tensor_reduce(
                            out=sb8[:, h * (CH // 2):(h + 1) * (CH // 2)],
                            in_=ps[:, 0:CH].rearrange("g (j s) -> g j s", s=2),
                            axis=mybir.AxisListType.X, op=mybir.AluOpType.add,
                        )
                    nc.vector.tensor_tensor(
                        out=sb8[:], in0=sb8[:], in1=w_c[:],
                        op=mybir.AluOpType.mult,
                    )
                    nc.vector.tensor_reduce(
                        out=outacc[:, ch * (CH // SLOT):(ch + 1) * (CH // SLOT)],
                        in_=sb8[:].rearrange("g (q i) -> g q i", i=SLOT),
                        axis=mybir.AxisListType.X, op=mybir.AluOpType.add,
                    )

            # combine across cores
            nc.sync.dma_start(out=partial[:], in_=outacc[:])
            nc.gpsimd.collective_compute(
                "AllReduce",
                mybir.AluOpType.add,
                replica_groups=[list(range(NCORES))],
                ins=[partial[:]],
                outs=[reduced[:]],
            )
            # out[n] = reduced[n % 8, n >> 3]; virtual partials to vout
            with nc.allow_non_contiguous_dma(reason="final relayout"):
                nc.sync.dma_start(
                    out=out[0:N, 0].rearrange("(q g) -> g q", g=G),
                    in_=reduced[:, 0:N // G],
                )
                nc.sync.dma_start(out=vout[:], in_=reduced[:, NREAL:npg])

    nc.finalize()
    return nc


def _get_nc(nch):
    if nch not in _cache:
        _cache[nch] = _build(nch)
    return _cache[nch]


def _build_timing(nch, passes=2):
    key = (nch, passes)
    if key not in _cache:
        _cache[key] = _build(nch, passes=passes)
    return _cache[key]


def _host_prep(x, edge_index, edge_weight, nch):
    x = np.asarray(x, dtype=np.float32).reshape(N)
    ei = np.asarray(edge_index)
    src = ei[0].astype(np.int64)
    dst = ei[1].astype(np.int32)
    w = np.asarray(edge_weight, dtype=np.float32)
    j = CH * nch

    # sort by src; rank within node; round-robin cores; node-slot columns
    order = np.argsort(src, kind="stable")
    s_s, d_s, w_s = src[order], dst[order], w[order]
    first = np.zeros(N + 1, np.int64)
    np.add.at(first, s_s + 1, 1)
    starts = np.cumsum(first)[:-1]
    rank = np.arange(E, dtype=np.int64) - starts[s_s]
    core = (rank % NCORES).astype(np.int32)
    i16r = rank // NCORES
    g = (s_s % G).astype(np.int32)
    col = (s_s >> 3) * SLOT + i16r
    # spill edges beyond SLOT per (node, core) to per-core virtual nodes
    npg = j // SLOT
    spill = i16r >= SLOT
    n_sp = int(spill.sum())
    assert n_sp <= (npg - NREAL) * G, f"too many spills: {n_sp}"
    vids = np.arange(n_sp, dtype=np.int64)
    g[spill] = vids % G
    col[spill] = (NREAL + vids // G) * SLOT
    core[spill] = 0
    vsrc = s_s[spill.nonzero()[0]]
    assert col.max() < j

    xtab = np.tile(
        np.ascontiguousarray(
            x.reshape(NE, 16, 2).transpose(1, 0, 2).reshape(16, 2 * NE)
        ),
        (8, 1),
    )
    iota2 = np.stack(
        [2 * (np.arange(128) % 16), 2 * (np.arange(128) % 16) + 1], axis=1
    ).astype(np.float32)
    bones = np.zeros((128, G), np.float32)
    for gg in range(G):
        bones[16 * gg:16 * (gg + 1), gg] = 1.0

    _host_prep.vsrc = vsrc
    in_maps = []
    for c in range(NCORES):
        m = core == c
        dc = np.zeros((G, j), np.int32)
        wc = np.zeros((G, j), np.float32)
        dc[g[m], col[m]] = d_s[m]
        wc[g[m], col[m]] = w_s[m]
        idx = (dc >> 5).astype(np.int16)            # [G, j]
        # wrap: dstw[16*gg + jj%16, jj//16] = idx[gg, jj]
        dstw = np.ascontiguousarray(
            idx.reshape(G, j // 16, 16).transpose(0, 2, 1)
        ).reshape(128, j // 16)
        in_maps.append({
            "dstw": dstw,
            "dlow": np.ascontiguousarray((dc & 31).astype(np.int8)),
            "wst": np.ascontiguousarray(wc),
            "xtab": xtab,
            "iota2": iota2,
            "bones": bones,
        })
    return in_maps


def kernel(x, edge_index, edge_weight, nch=NCH_FULL):
    from concourse.bass_utils import run_bass_kernel_spmd

    nc = _get_nc(nch)
    in_maps = _host_prep(x, edge_index, edge_weight, nch)
    res = run_bass_kernel_spmd(nc, in_maps, list(range(NCORES)))
    out = res.results[0]["out"].astype(np.float32).reshape(N, 1)
    vout = res.results[0]["vout"].astype(np.float32)
    vsrc = _host_prep.vsrc
    vvals = vout.T.reshape(-1)[:len(vsrc)]
    np.add.at(out[:, 0], vsrc, vvals)
    return out
